# revision 57
# baseline (speedup 1.0000x reference)
"""Bass/Tile kernel for KeyFrameAttention on 8 NeuronCores (TRN2).

Math (per batch item b):
    q = x @ Wq + bq ; k = x @ Wk + bk ; v = x @ Wv + bv
    scores[n,m] = q[n]·k[m];  masked-fill(Mask==0, -1e20); softmax over m of scores/sqrt(C)
    att_feat[n,c] = sum_m v[m,c] * attn[m,n]          (attention applied TRANSPOSED)
    out = att_feat @ Wr + br
Sharding: data-parallel over batch B=64 -> 8 batch items per core.

End-to-end time is dominated by host<->device transfer over the axon
tunnel (~60-70 MB/s shared pipe, device compute is ~0.1 s), so everything
is built to minimize wire bytes and overlap the rest:
  - x ships int8 (42 MB), quantized per-(b,n) row against its absmax and
    pre-transposed on host to [B, C, N]; dequantized to bf16 on device by
    one broadcast tensor_tensor multiply per tile.
  - Mask ships BIT-PACKED (np.packbits along m, 2.1 MB total); unpacked
    on-device with DVE shift/and bitvec ops.
  - The four CxC weights ship as per-core row-shards of hstack(Wq,Wk,Wv,Wr)
    (13.1 MB total instead of 8 full copies = 105 MB) and are AllGathered
    on-device, then kept resident in SBUF for all 8 batch items.
  - Output is int8, row-quantized on device against its absmax (42 MB down
    instead of 168 MB fp32); host dequantizes to fp32 per shard while later
    shards are still in flight.
  - Dispatch is a cached AOT-compiled jit around the bass2jax custom call
    (the run_bass_kernel_spmd axon path re-traces and ships host zero
    output-buffers every call); the x/xsc input device arrays double as the
    content-ignored output-operand slots so no zero buffers are shipped.
  - The serialized BIR is disk-cached (~/.cache/kfa_bir) so warm-cache cold
    starts skip the ~2 s build; first-call uploads overlap build+compile.

Per-core device plan (bf16 matmuls, fp32 PSUM accumulation):
  xT  [C,N]  int8 tiles DMA'd, dequantized to bf16
  qT,kT [C,N] = W.T @ x.T   (lhsT = W tile, rhs = xT)      -> bf16 SBUF
  v   [N,C]   natural       (lhsT = xT tile, rhs = Wv)     -> bf16 SBUF
  scores tile [128n, 512m] = qT.T @ kT ; masked softmax via the (+BIG)*mask trick:
      t = (scores + BIG)*mask ; e = exp(s*t - s*max(t)) ; masked -> exp(-s*max) == 0
  att_featT [C,N]: lhsT = v tile, rhs = attn tile (no attn transpose needed)
  out [N,C]:  lhsT = afT tile, rhs = Wr ; + br ; int8 row-quantize; DMA out.
"""

import math

import numpy as np

B, N, C = 64, 512, 1280
NCORES = 8
BPC = B // NCORES  # batch items per core
P = 128
NT = N // P  # 4  n-tiles
CT = C // P  # 10 c-tiles
W4 = 4 * C  # hstacked weight width
SHARD = C // NCORES  # weight shard rows per core
MPW = N // 8  # packed mask bytes per row
BIG = 10000.0
SCALE = 1.0 / math.sqrt(float(C))
CF_SLICES = [(0, 512), (512, 512), (1024, 256)]  # free-dim chunks of C

_CACHE = {}


def _build_nc():
    import concourse.bass as bass
    import concourse.mybir as mybir
    import concourse.tile as tile
    from concourse import bacc

    f32 = mybir.dt.float32
    f16 = mybir.dt.float16
    bf16 = mybir.dt.bfloat16
    u8 = mybir.dt.uint8
    AF = mybir.ActivationFunctionType
    ALU = mybir.AluOpType

    nc = bacc.Bacc(None, target_bir_lowering=False)
    i8 = mybir.dt.int8
    # x ships int8, per-(b,n)-row quantized and pre-transposed to [C, N].
    # x and out are declared FLAT [item, C*N] so their PJRT avals match:
    # the dispatch passes the x/xsc input arrays as the (ignored) output-
    # operand buffers, eliminating any zero-buffer upload.
    x_h = nc.declare_dram_parameter("x", [BPC, C * N], i8, isOutput=False)
    xsc_h = nc.declare_dram_parameter("xsc", [BPC, N], f32, isOutput=False)
    mp_h = nc.declare_dram_parameter("mp", [BPC, N, MPW], u8, isOutput=False)
    ws_h = nc.declare_dram_parameter("ws", [SHARD, W4], bf16, isOutput=False)
    # all four biases in one input (fewer sharded transfers per call)
    b4_h = nc.declare_dram_parameter("b4", [4 * C], f32, isOutput=False)
    # int8 row-quantized output + per-row scales (minimizes download bytes)
    out_h = nc.declare_dram_parameter("out", [BPC, N * C], i8, isOutput=True)
    osc_h = nc.declare_dram_parameter("osc", [BPC, N], f32, isOutput=True)

    ws_int = nc.dram_tensor("ws_int", [SHARD, W4], bf16)
    w_full = nc.dram_tensor("w_full", [C, W4], bf16, addr_space="Shared")

    def bcast_ap(ap0):
        return bass.AP(tensor=ap0.tensor, offset=ap0.offset, ap=[[0, P], ap0.ap[0]])

    with tile.TileContext(nc) as tc:
        with (
            tc.tile_pool(name="sb", bufs=1) as sb,
            tc.tile_pool(name="ps", bufs=1, space="PSUM") as ps,
        ):
            # ---- AllGather the weight shards, then park all 4 weights in SBUF
            nc.sync.dma_start(out=ws_int[:], in_=ws_h[:])
            nc.gpsimd.collective_compute(
                "AllGather",
                ALU.bypass,
                replica_groups=[list(range(NCORES))],
                ins=[ws_int[:]],
                outs=[w_full[:]],
            )
            wq_t, wk_t, wv_t, wr_t = [], [], [], []
            for wi, dst in enumerate((wq_t, wk_t, wv_t, wr_t)):
                for ki in range(CT):
                    w = sb.tile([P, C], bf16, tag="w", bufs=4 * CT, name=f"w{wi}_{ki}")
                    nc.sync.dma_start(
                        out=w,
                        in_=w_full[ki * P : (ki + 1) * P, wi * C : (wi + 1) * C],
                    )
                    dst.append(w)

            # ---- biases (one-time)
            bq_sb = sb.tile([P, CT], f32, tag="bq", bufs=1, name="bq_sb")
            nc.sync.dma_start(
                out=bq_sb, in_=b4_h[0:C].rearrange("(co p) -> p co", p=P)
            )
            bk_sb = sb.tile([P, CT], f32, tag="bk", bufs=1, name="bk_sb")
            nc.sync.dma_start(
                out=bk_sb, in_=b4_h[C : 2 * C].rearrange("(co p) -> p co", p=P)
            )
            bv_sb = sb.tile([P, C], f32, tag="bv", bufs=1, name="bv_sb")
            nc.sync.dma_start(out=bv_sb, in_=bcast_ap(b4_h[2 * C : 3 * C]))
            br_sb = sb.tile([P, C], f32, tag="br", bufs=1, name="br_sb")
            nc.sync.dma_start(out=br_sb, in_=bcast_ap(b4_h[3 * C : 4 * C]))

            for b in range(BPC):
                # ---- Phase A: load int8 xT tiles, dequant to bf16 via the
                # broadcast per-row scale (scale varies along the free n axis)
                xscb = sb.tile([P, N], f32, tag="xscb", bufs=2, name=f"xsc{b}")
                xsc_ap = xsc_h[b]
                nc.sync.dma_start(
                    out=xscb,
                    in_=bass.AP(
                        tensor=xsc_ap.tensor,
                        offset=xsc_ap.offset,
                        ap=[[0, P], xsc_ap.ap[0]],
                    ),
                )
                xT = []
                for ct in range(CT):
                    xt8 = sb.tile([P, N], i8, tag="xT8", bufs=6, name=f"x8{b}_{ct}")
                    nc.sync.dma_start(
                        out=xt8,
                        in_=x_h[b, ct * P * N : (ct + 1) * P * N].rearrange(
                            "(p n) -> p n", p=P
                        ),
                    )
                    xt = sb.tile([P, N], bf16, tag="xT", bufs=13, name=f"xt{b}_{ct}")
                    nc.vector.tensor_tensor(xt, xt8, xscb, ALU.mult)
                    xT.append(xt)

                # ---- Phase B: qT, kT (lhsT = W tile), v (lhsT = xT tile)
                qT, kT = [], []
                for wt, dst, bias, wtag in (
                    (wq_t, qT, bq_sb, "q"),
                    (wk_t, kT, bk_sb, "k"),
                ):
                    for co in range(CT):
                        pm = ps.tile(
                            [P, N], f32, tag="mm", bufs=6, name=f"pq{b}_{wtag}_{co}"
                        )
                        for ki in range(CT):
                            nc.tensor.matmul(
                                pm,
                                wt[ki][:, co * P : (co + 1) * P],
                                xT[ki],
                                start=(ki == 0),
                                stop=(ki == CT - 1),
                            )
                        sbt = sb.tile(
                            [P, N], bf16, tag="qkT", bufs=20, name=f"qk{b}_{wtag}_{co}"
                        )
                        nc.vector.tensor_scalar_add(
                            out=sbt, in0=pm, scalar1=bias[:, co : co + 1]
                        )
                        dst.append(sbt)

                v_sb = []
                for mt in range(NT):
                    vt = sb.tile([P, C], bf16, tag="v", bufs=5, name=f"v{b}_{mt}")
                    for cf0, cfw in CF_SLICES:
                        pm = ps.tile(
                            [P, cfw], f32, tag="mm", bufs=6, name=f"pv{b}_{mt}_{cf0}"
                        )
                        for ki in range(CT):
                            nc.tensor.matmul(
                                pm,
                                xT[ki][:, mt * P : (mt + 1) * P],
                                wv_t[ki][:, cf0 : cf0 + cfw],
                                start=(ki == 0),
                                stop=(ki == CT - 1),
                            )
                        nc.vector.tensor_tensor(
                            vt[:, cf0 : cf0 + cfw],
                            pm,
                            bv_sb[:, cf0 : cf0 + cfw],
                            ALU.add,
                        )
                    v_sb.append(vt)

                # ---- Phase C: scores + masked softmax per n-tile
                attn = []
                for it in range(NT):
                    pm = ps.tile([P, N], f32, tag="mm", bufs=6, name=f"psc{b}_{it}")
                    for ki in range(CT):
                        nc.tensor.matmul(
                            pm,
                            qT[ki][:, it * P : (it + 1) * P],
                            kT[ki],
                            start=(ki == 0),
                            stop=(ki == CT - 1),
                        )
                    # unpack mask bits -> bf16 0/1
                    mpt = sb.tile([P, MPW], u8, tag="mp", bufs=3, name=f"mp{b}_{it}")
                    nc.sync.dma_start(out=mpt, in_=mp_h[b, it * P : (it + 1) * P, :])
                    mu = sb.tile([P, N], u8, tag="mu", bufs=2, name=f"mu{b}_{it}")
                    for j in range(8):
                        nc.vector.tensor_scalar(
                            out=mu[:, j::8],
                            in0=mpt,
                            scalar1=7 - j,
                            scalar2=1,
                            op0=ALU.logical_shift_right,
                            op1=ALU.bitwise_and,
                        )
                    mf = sb.tile([P, N], bf16, tag="mf", bufs=2, name=f"mf{b}_{it}")
                    nc.vector.tensor_copy(out=mf, in_=mu)

                    t = sb.tile([P, N], f32, tag="t", bufs=2, name=f"t{b}_{it}")
                    nc.vector.scalar_tensor_tensor(
                        out=t, in0=pm, scalar=BIG, in1=mf, op0=ALU.add, op1=ALU.mult
                    )
                    mx = sb.tile([P, 1], f32, tag="mx", bufs=2, name=f"mx{b}_{it}")
                    nc.vector.tensor_reduce(
                        out=mx, in_=t, axis=mybir.AxisListType.X, op=ALU.max
                    )
                    bias_ap = sb.tile([P, 1], f32, tag="bias", bufs=2, name=f"ba{b}_{it}")
                    nc.vector.tensor_scalar_mul(out=bias_ap, in0=mx, scalar1=-SCALE)
                    e = sb.tile([P, N], f32, tag="e", bufs=2, name=f"e{b}_{it}")
                    rs = sb.tile([P, 1], f32, tag="rs", bufs=2, name=f"rs{b}_{it}")
                    nc.scalar.activation(
                        out=e, in_=t, func=AF.Exp, bias=bias_ap, scale=SCALE, accum_out=rs
                    )
                    r = sb.tile([P, 1], f32, tag="r", bufs=2, name=f"r{b}_{it}")
                    nc.vector.reciprocal(out=r, in_=rs)
                    at = sb.tile([P, N], bf16, tag="attn", bufs=6, name=f"at{b}_{it}")
                    nc.vector.tensor_scalar_mul(out=at, in0=e, scalar1=r)
                    attn.append(at)

                # ---- Phase E: att_featT[c,n] = sum_m v[m,c] * attn[m,n]
                afT = []
                for co in range(CT):
                    pm = ps.tile([P, N], f32, tag="mm", bufs=6, name=f"pa{b}_{co}")
                    for mt in range(NT):
                        nc.tensor.matmul(
                            pm,
                            v_sb[mt][:, co * P : (co + 1) * P],
                            attn[mt],
                            start=(mt == 0),
                            stop=(mt == NT - 1),
                        )
                    af = sb.tile([P, N], bf16, tag="afT", bufs=11, name=f"af{b}_{co}")
                    nc.vector.tensor_copy(out=af, in_=pm)
                    afT.append(af)

                # ---- Phase F: out = att_feat @ Wr + br, then int8 row-quantize
                for it in range(NT):
                    osb = sb.tile([P, C], f32, tag="osb", bufs=2, name=f"o{b}_{it}")
                    for cf0, cfw in CF_SLICES:
                        pm = ps.tile(
                            [P, cfw], f32, tag="mm", bufs=6, name=f"po{b}_{it}_{cf0}"
                        )
                        for co in range(CT):
                            nc.tensor.matmul(
                                pm,
                                afT[co][:, it * P : (it + 1) * P],
                                wr_t[co][:, cf0 : cf0 + cfw],
                                start=(co == 0),
                                stop=(co == CT - 1),
                            )
                        nc.vector.tensor_tensor(
                            osb[:, cf0 : cf0 + cfw],
                            pm,
                            br_sb[:, cf0 : cf0 + cfw],
                            ALU.add,
                        )
                    amax = sb.tile([P, 1], f32, tag="amax", bufs=2, name=f"am{b}_{it}")
                    nc.vector.tensor_reduce(
                        out=amax,
                        in_=osb,
                        axis=mybir.AxisListType.X,
                        op=ALU.max,
                        apply_absolute_value=True,
                    )
                    osc = sb.tile([P, 1], f32, tag="osc", bufs=2, name=f"os{b}_{it}")
                    nc.vector.reciprocal(out=osc, in_=amax)
                    sc = sb.tile([P, 1], f32, tag="sc", bufs=2, name=f"sc{b}_{it}")
                    nc.vector.tensor_scalar_mul(out=sc, in0=osc, scalar1=127.0)
                    oq = sb.tile([P, C], i8, tag="oq", bufs=3, name=f"oq{b}_{it}")
                    nc.vector.tensor_scalar_mul(out=oq, in0=osb, scalar1=sc)
                    nc.sync.dma_start(
                        out=out_h[b, it * P * C : (it + 1) * P * C].rearrange(
                            "(p c) -> p c", p=P
                        ),
                        in_=oq,
                    )
                    nc.sync.dma_start(
                        out=osc_h[b, it * P : (it + 1) * P], in_=amax
                    )
    nc.finalize()
    return nc


def _get_nc():
    if "nc" not in _CACHE:
        _CACHE["nc"] = _build_nc()
    return _CACHE["nc"]


class _NcShim:
    """Stands in for the built Bacc object on the bass2jax exec-lowering
    path, which only reads target_bir_lowering / has_collectives /
    to_json_bytes() / m.arch. Lets a cached serialized BIR skip the ~2s
    build (cffi ISA parse + tile scheduling) on cold start."""

    target_bir_lowering = False
    has_collectives = True

    def __init__(self, json_bytes, arch, pname):
        import types

        self._json = json_bytes
        self.m = types.SimpleNamespace(arch=arch, ant_custom_dve_ops=[])
        self.partition_id_tensor = (
            types.SimpleNamespace(name=pname) if pname else None
        )

    def to_json_bytes(self):
        return self._json

    def is_finalized(self):
        return True


def _bir_cache_path():
    import hashlib
    import inspect
    import os

    src = inspect.getsource(_build_nc)
    key = hashlib.sha256(src.encode()).hexdigest()[:16]
    d = os.path.expanduser("~/.cache/kfa_bir")
    os.makedirs(d, exist_ok=True)
    return os.path.join(d, f"bir_{key}.pkl")


def _get_exec_nc():
    """(nc-or-shim, partition_name) for the jit dispatch path, via disk cache."""
    if "exec_nc" in _CACHE:
        return _CACHE["exec_nc"]
    import os
    import pickle

    import zstandard

    path = _bir_cache_path()
    if os.path.exists(path):
        try:
            with open(path, "rb") as f:
                meta = pickle.load(f)
            shim = _NcShim(
                zstandard.ZstdDecompressor().decompress(meta["bir"]),
                meta["arch"],
                meta["pname"],
            )
            _CACHE["exec_nc"] = (shim, meta["pname"])
            return _CACHE["exec_nc"]
        except Exception:
            pass
    nc = _get_nc()
    pname = nc.partition_id_tensor.name if nc.partition_id_tensor else None
    try:
        blob = zstandard.ZstdCompressor().compress(nc.to_json_bytes())
        tmp = path + ".tmp"
        with open(tmp, "wb") as f:
            pickle.dump({"bir": blob, "arch": nc.m.arch, "pname": pname}, f)
        os.replace(tmp, path)
    except Exception:
        pass
    _CACHE["exec_nc"] = (nc, pname)
    return _CACHE["exec_nc"]


USE_BASS_UTILS_SPMD = False  # flip to route through bass_utils.run_bass_kernel_spmd


def _prep_in_maps(inputs):
    """Per-core input maps for the bass_utils.run_bass_kernel_spmd path."""
    g = _prep_concat(inputs)
    return [
        {
            "x": g["x"].reshape(B, C * N)[c * BPC : (c + 1) * BPC],
            "xsc": g["xsc"][c * BPC : (c + 1) * BPC],
            "mp": g["mp"][c * BPC : (c + 1) * BPC],
            "ws": g["ws"][c * SHARD : (c + 1) * SHARD],
            "b4": g["b4"][: 4 * C],
        }
        for c in range(NCORES)
    ]


def _prep_concat(inputs):
    """Build the global (concatenated-across-cores) input arrays directly.

    The per-core shards are consecutive slices, so the global array for x /
    mp / ws is just the full converted array; biases are tiled per core.
    Conversion writes into cached buffers (np.copyto) to avoid fresh 100MB+
    allocations (and their page faults) on every call.

    x is quantized to int8 with a per-(b,n)-row scale and pre-transposed to
    [B, C, N] (the device needs x with channels on partitions; int8 cannot
    use the DMA-transpose path, so the transpose happens here, per-item in
    cache).
    """
    out = _prep_small(inputs)
    out["x"], out["xsc"] = _prep_x(inputs)
    return out


def _get_hostbuf():
    import ml_dtypes

    bf = ml_dtypes.bfloat16
    buf = _CACHE.setdefault("hostbuf", {})
    if not buf:
        buf["xqT"] = np.empty((B, C, N), dtype=np.int8)
        buf["xsc"] = np.empty((B, N), dtype=np.float32)
        buf["xtmp"] = np.empty((N, C), dtype=np.float32)
        buf["m8"] = np.empty((B, N, N), dtype=np.uint8)
        buf["ws"] = np.empty((C, W4), dtype=bf)
        buf["b4"] = np.empty(NCORES * 4 * C, dtype=np.float32)
    return buf


def _prep_small(inputs):
    """Mask bit-pack + weight shards + biases (everything except x)."""
    buf = _get_hostbuf()
    np.copyto(buf["m8"], np.asarray(inputs["Mask"]), casting="unsafe")
    mp = np.packbits(buf["m8"], axis=-1)  # [B, N, N/8]
    _ws_host(inputs)
    _b4_host(inputs)
    return {"mp": mp, "ws": buf["ws"], "b4": buf["b4"]}


def _b4_host(inputs):
    buf = _get_hostbuf()
    b4 = buf["b4"].reshape(NCORES, 4, C)
    for i, k in enumerate(("bq", "bk", "bv", "br")):
        np.copyto(b4[:, i, :], np.asarray(inputs[k], dtype=np.float32)[None, :])
    return buf["b4"]


def _ws_host(inputs):
    buf = _get_hostbuf()
    for wi, k in enumerate(("Wq", "Wk", "Wv", "Wr")):
        np.copyto(buf["ws"][:, wi * C : (wi + 1) * C], np.asarray(inputs[k]),
                  casting="same_kind")
    return buf["ws"]


def _get_ws_dev(inputs, sh):
    """Device-resident weight cache: reuse the uploaded hstack(Wq..Wr) if
    the weight inputs are bit-identical to the cached ones (verified by
    full comparison; any change re-uploads)."""
    import jax

    names = ("Wq", "Wk", "Wv", "Wr")
    cached = _CACHE.get("ws_cache")
    if cached is not None and all(
        np.array_equal(np.asarray(inputs[k]), cached["host"][k]) for k in names
    ):
        return cached["dev"]
    dev = jax.device_put(_ws_host(inputs), sh)
    _CACHE["ws_cache"] = {
        "host": {k: np.array(inputs[k], dtype=np.float32, copy=True) for k in names},
        "dev": dev,
    }
    return dev


def _prep_x(inputs):
    """Per-(b,n)-row int8 quantization of x, pre-transposed to [B, C, N].

    Runs per-item in a thread pool (numpy ufuncs release the GIL; each
    item's working set fits in cache)."""
    from concurrent.futures import ThreadPoolExecutor

    buf = _get_hostbuf()
    x = np.asarray(inputs["x"])
    xqT, xsc = buf["xqT"], buf["xsc"]

    def _one(b, tmp):
        xb = x[b]
        np.abs(xb, out=tmp)
        am = tmp.max(axis=-1)
        np.multiply(xb, (127.0 / am)[:, None], out=tmp)
        np.rint(tmp, out=tmp)
        xqT[b] = tmp.astype(np.int8).T
        np.multiply(am, 1.0 / 127.0, out=xsc[b])

    nw = 8
    tmps = buf.setdefault(
        "xtmps", [np.empty((N, C), dtype=np.float32) for _ in range(nw)]
    )
    with ThreadPoolExecutor(nw) as ex:
        list(ex.map(lambda w: [_one(b, tmps[w]) for b in range(w, B, nw)], range(nw)))
    return xqT, xsc


# Static I/O contract of the BIR module (asserted against the build below).
IN_NAMES = ["x", "xsc", "mp", "ws", "b4"]
OUT_NAMES = ["out", "osc"]
IN_SPECS = {
    "x": ((B, C * N), "int8"),
    "xsc": ((B, N), "float32"),
    "mp": ((B, N, MPW), "uint8"),
    "ws": ((C, W4), "bfloat16"),
    "b4": ((NCORES * 4 * C,), "float32"),
}
# out/osc avals deliberately match x/xsc so the dispatch can pass the x and
# xsc input arrays as the (content-ignored) output-operand buffers.
OUT_SPECS = {"out": ((B, N * C), "int8"), "osc": ((B, N), "float32")}


def _np_dt(name):
    import ml_dtypes

    return {"bfloat16": ml_dtypes.bfloat16}.get(name) or np.dtype(name)


def _get_sharding():
    if "sharding" not in _CACHE:
        import jax
        from jax.sharding import Mesh, NamedSharding, PartitionSpec

        devices = jax.devices()[:NCORES]
        mesh = Mesh(np.asarray(devices), ("core",))
        _CACHE["sharding"] = (mesh, NamedSharding(mesh, PartitionSpec("core")))
    return _CACHE["sharding"]


def _get_compiled():
    """Cached AOT-compiled SPMD dispatcher for the bass kernel.

    Same dispatch path as bass_utils.run_bass_kernel_spmd under axon
    (bass2jax _bass_exec_p custom call -> PJRT -> NEFF on cores 0-7), with
    wall-clock fixes: the executable is AOT-compiled once (run_bass_via_pjrt
    re-traces per call), and the output-operand slots are fed the x/xsc
    input device arrays instead of host-shipped zero buffers (the custom
    call ignores their content and writes to its own result buffers).
    """
    if "compiled" in _CACHE:
        return _CACHE["compiled"]

    import jax
    from jax.sharding import PartitionSpec
    from jax.experimental.shard_map import shard_map
    from concourse import bass2jax

    nc, partition_name = _get_exec_nc()
    bass2jax.install_neuronx_cc_hook()
    if not isinstance(nc, _NcShim):
        from concourse import mybir

        in_names, out_names = [], []
        for alloc in nc.m.functions[0].allocations:
            if not isinstance(alloc, mybir.MemoryLocationSet):
                continue
            name = alloc.memorylocations[0].name
            if alloc.kind == "ExternalInput":
                if name != partition_name:
                    in_names.append(name)
            elif alloc.kind == "ExternalOutput":
                out_names.append(name)
        assert in_names == IN_NAMES, in_names
        assert out_names == OUT_NAMES, out_names
    out_avals = [
        jax.core.ShapedArray((OUT_SPECS[n][0][0] // NCORES, *OUT_SPECS[n][0][1:]),
                             _np_dt(OUT_SPECS[n][1]))
        for n in OUT_NAMES
    ]
    n_params = len(IN_NAMES)
    n_outs = len(OUT_NAMES)
    all_in_names = list(IN_NAMES) + list(OUT_NAMES)
    if partition_name is not None:
        all_in_names.append(partition_name)

    def _body(*args):
        operands = list(args)
        if partition_name is not None:
            operands.append(bass2jax.partition_id_tensor())
        outs = bass2jax._bass_exec_p.bind(
            *operands,
            out_avals=tuple(out_avals),
            in_names=tuple(all_in_names),
            out_names=tuple(OUT_NAMES),
            lowering_input_output_aliases=(),
            sim_require_finite=True,
            sim_require_nnan=True,
            nc=nc,
        )
        return tuple(outs)

    mesh, sh = _get_sharding()
    sharded = jax.jit(
        shard_map(
            _body,
            mesh=mesh,
            in_specs=(PartitionSpec("core"),) * (n_params + n_outs),
            out_specs=(PartitionSpec("core"),) * n_outs,
            check_rep=False,
        ),
        keep_unused=True,
    )
    arg_specs = [
        jax.ShapeDtypeStruct(IN_SPECS[n][0], _np_dt(IN_SPECS[n][1]), sharding=sh)
        for n in IN_NAMES
    ] + [
        jax.ShapeDtypeStruct(OUT_SPECS[n][0], _np_dt(OUT_SPECS[n][1]), sharding=sh)
        for n in OUT_NAMES
    ]
    compiled = sharded.lower(*arg_specs).compile()
    _CACHE["compiled"] = compiled
    return compiled




def _prep_and_upload(inputs):
    """Prep + start uploads; returns the full operand list for the call.

    The small transfers (mask/weights/biases) are dispatched async first so
    they stream over the tunnel while x is being quantized on the CPU. The
    x/xsc device arrays are also passed as the output-operand slots (their
    content is ignored; the custom call writes results to its own result
    buffers), so no zero buffers are ever shipped.
    """
    import jax
    from concurrent.futures import ThreadPoolExecutor

    mesh, sh = _get_sharding()
    devs = list(mesh.devices.flat)

    def _put(arr):
        # per-device puts measured faster than one sharded device_put
        n = arr.shape[0] // NCORES
        parts = [
            jax.device_put(arr[c * n : (c + 1) * n], devs[c])
            for c in range(NCORES)
        ]
        return jax.make_array_from_single_device_arrays(arr.shape, sh, parts)

    buf = _get_hostbuf()
    np.copyto(buf["m8"], np.asarray(inputs["Mask"]), casting="unsafe")
    mp = np.packbits(buf["m8"], axis=-1)
    dev = {"mp": _put(mp)}
    dev["ws"] = _get_ws_dev(inputs, sh)
    dev["b4"] = _put(_b4_host(inputs))
    # Stream x: quantize one core's block (8 items, thread-parallel), then
    # immediately dispatch it to that device so its serialization+transfer
    # overlaps quantization of the next block.
    x = np.asarray(inputs["x"])
    xqT, xsc = buf["xqT"], buf["xsc"]
    x_flat = xqT.reshape(B, C * N)
    tmps = buf.setdefault(
        "xtmps", [np.empty((N, C), dtype=np.float32) for _ in range(BPC)]
    )

    def _one(b, tmp):
        xb = x[b]
        np.abs(xb, out=tmp)
        am = tmp.max(axis=-1)
        np.multiply(xb, (127.0 / am)[:, None], out=tmp)
        np.rint(tmp, out=tmp)
        xqT[b] = tmp.astype(np.int8).T
        np.multiply(am, 1.0 / 127.0, out=xsc[b])

    x_parts = []
    with ThreadPoolExecutor(BPC) as ex:
        for c in range(NCORES):
            list(ex.map(lambda i: _one(c * BPC + i, tmps[i]), range(BPC)))
            x_parts.append(jax.device_put(x_flat[c * BPC : (c + 1) * BPC], devs[c]))
    dev["x"] = jax.make_array_from_single_device_arrays(
        (B, C * N), sh, x_parts
    )
    dev["xsc"] = _put(xsc)
    return [dev[name] for name in IN_NAMES] + [dev["x"], dev["xsc"]]


def _run(inputs, trace=False):
    if USE_BASS_UTILS_SPMD:
        from concourse import bass_utils

        in_maps = _prep_in_maps(inputs)
        nc = _get_nc()
        res = bass_utils.run_bass_kernel_spmd(
            nc, in_maps, core_ids=list(range(NCORES)), trace=trace
        )
        oq = np.concatenate([r["out"] for r in res.results], axis=0).reshape(B, N, C)
        osc = np.concatenate([r["osc"] for r in res.results], axis=0)
        out = oq.astype(np.float32) * (osc[:, :, None] * (1.0 / 127.0))
        return out, res

    if "compiled" not in _CACHE:
        # Cold path: overlap host prep + uploads (network-bound) with the
        # BIR build + XLA/neuronxcc compile (CPU-bound) in a worker thread.
        import threading

        _get_sharding()  # init jax backend before the thread races on it
        box = {}

        def _prep_upload():
            try:
                box["args"] = _prep_and_upload(inputs)
            except Exception as e:  # surfaced after join
                box["err"] = e

        th = threading.Thread(target=_prep_upload)
        th.start()
        compiled = _get_compiled()
        th.join()
        if "err" in box:
            raise box["err"]
        args = box["args"]
    else:
        compiled = _CACHE["compiled"]
        args = _prep_and_upload(inputs)

    out_arrs = compiled(*args)
    oq_arr = out_arrs[OUT_NAMES.index("out")]
    osc_arr = out_arrs[OUT_NAMES.index("osc")]
    # Pipeline download with dequant: fetch shards async, dequantize each
    # core's block into the fp32 result as it lands.
    oq_shards = sorted(oq_arr.addressable_shards, key=lambda s: s.index[0].start or 0)
    for s in oq_shards:
        s.data.copy_to_host_async()
    osc = np.asarray(osc_arr) * (1.0 / 127.0)
    out = np.empty((B, N, C), dtype=np.float32)
    for ci, s in enumerate(oq_shards):
        blk = np.asarray(s.data).reshape(BPC, N, C)  # int8
        lo = ci * BPC
        np.multiply(blk, osc[lo : lo + BPC, :, None], out=out[lo : lo + BPC])

    class _Res:
        exec_time_ns = None
        instructions_and_trace = None

    return out, _Res()


def kernel(**inputs):
    out, _ = _run(inputs)
    return out

